# revision 19
# baseline (speedup 1.0000x reference)
"""Trainium2 Bass kernel for nn_NodeModel (GNN message passing + 3-layer node MLP).

v2 strategy (node-parallel, 8 cores, 512-node supertiles):
  - Host: sort edges by destination, bucket into 128-node tiles, pad each
    tile's edge list to K chunks of 128 edges. 100 tiles/core, grouped into
    25 supertiles of 4 tiles (512 nodes).
  - Device per supertile:
      agg:   one-hot via DVE/GPSIMD is_equal built [128,512]-wide (4 chunks
             per op via strided/broadcast APs), matmul-accumulated per tile.
      MLP:   z computed col-major [h, n] with W-stationary 512-wide matmuls.
             LayerNorm stats on the PE: per-chunk matmuls with zb/zsq as the
             stationary operand against +-ones/H vectors give -mu and E[z^2]
             as [node,1] PSUM columns; small-ops run on [128,4] tiles.
             Per-node normalize is fused into a per-chunk ACT Exp
             (scale=rstd).  gamma is folded into the forward transpose as a
             diag(g) rhs; the -mu*rstd x g term is added by a rank-1 matmul;
             beta is folded into the final Ln via per-partition EB=0.5*e^be
             scale: act_next = ln(EB * exp(g*(z-mu)*rstd) + 0.5) == ssp out.
  - Output returned bf16 from device, cast to f32 on host.
"""

import os
import sys

import numpy as np

sys.path.insert(0, "/opt/trn_rl_repo")

import bass_rust as _bass_rust
import ml_dtypes

from concourse import bacc, bass, hw_specs, mybir
from concourse import tile as tile_mod
from concourse.bass_utils import run_bass_kernel_spmd
from concourse.masks import make_identity


class _Bacc(bacc.Bacc):
    """Bacc with the ACT table chooser pinned to natural_log_exp_and_others
    (holds Ln+Exp+Identity+Copy+Square), avoiding ~1.3us table swaps."""

    def insert_act_table_loads(self):
        has_activation = any(
            isinstance(i, mybir.InstActivation)
            for b in self.main_func.blocks
            for i in b.instructions
        )
        if not has_activation:
            return
        keep = "natural_log_exp_and_others"
        tables = [
            (n, (s if n == keep else set()))
            for n, s in hw_specs.get_activation_tables(self.m.arch).items()
        ]
        _bass_rust.insert_act_table_loads(self, tables)


N, E, H = 100000, 600000, 128
NC = 8
P = 128
TPC = 100                # 128-node tiles per core
ST = 4                   # tiles per supertile
NST = TPC // ST          # supertiles per core (25)
SW = ST * P              # supertile width in nodes (512)
NPC = TPC * P            # nodes per core (12800)
NPAD = NPC * NC          # padded node count (102400)
NT = NPAD // P           # total node tiles (800)

F32 = mybir.dt.float32
BF16 = mybir.dt.bfloat16

LAST_RESULT = None  # BassKernelResults of the most recent run (for profiling)


def _host_prep(x, edge_index, edge_attr):
    col = np.asarray(edge_index)[1].astype(np.int64)
    ea = np.ascontiguousarray(np.asarray(edge_attr, dtype=np.float32))
    order = np.argsort(col, kind="stable")
    col_s = col[order]
    tile_of = col_s >> 7
    counts = np.bincount(tile_of, minlength=NT)
    K = int(np.ceil(counts.max() / P))
    S = K * P
    starts = np.zeros(NT + 1, np.int64)
    starts[1:] = np.cumsum(counts)
    pos = np.arange(E) - starts[tile_of]
    slot = tile_of * S + pos
    slot_edge = np.zeros(NT * S, np.int64)
    slot_edge[slot] = order
    col_local = np.full(NT * S, 128.0, np.float32)
    col_local[slot] = (col_s & 127).astype(np.float32)
    payload = ea[slot_edge]  # [NT*S, H]

    x_pad = np.zeros((NPAD, H), np.float32)
    x_pad[:N] = np.asarray(x, dtype=np.float32)

    per_core = []
    for c in range(NC):
        r0, r1 = c * TPC * S, (c + 1) * TPC * S
        # edges: [NST*P, ST*K*P] bf16; row = st*128+e, col = (t*K+k)*128+h
        ed_c = np.ascontiguousarray(
            payload[r0:r1]
            .reshape(NST, ST, K, P, H)
            .transpose(0, 3, 1, 2, 4)
            .reshape(NST * P, ST * K * H)
            .astype(ml_dtypes.bfloat16)
        )
        # cols: [P, NST*ST*K] bf16; col index = st*ST*K + t*K + k
        col_c = np.ascontiguousarray(
            col_local[r0:r1]
            .reshape(NST, ST, K, P)
            .transpose(3, 0, 1, 2)
            .reshape(P, NST * ST * K)
            .astype(ml_dtypes.bfloat16)
        )
        # xt: [NST*P, SW] bf16 col-major per supertile; row st*128+h, col t*128+n
        xt_c = np.ascontiguousarray(
            x_pad[c * NPC : (c + 1) * NPC]
            .reshape(NST, ST, P, H)
            .transpose(0, 3, 1, 2)
            .reshape(NST * P, SW)
            .astype(ml_dtypes.bfloat16)
        )
        per_core.append((ed_c, col_c, xt_c))
    return K, per_core


def _build_program(K):
    nc = _Bacc("TRN2", target_bir_lowering=False, debug=False, num_devices=NC)

    ed_h = nc.dram_tensor("edges", [NST * P, ST * K * P], BF16, kind="ExternalInput")
    cols_h = nc.dram_tensor("cols", [P, NST * ST * K], BF16, kind="ExternalInput")
    xt_h = nc.dram_tensor("xt", [NST * P, SW], BF16, kind="ExternalInput")
    w_h = {
        name: nc.dram_tensor(name, [P, P], BF16, kind="ExternalInput")
        for name in ("w1a", "w1b", "w2", "w3")
    }
    diag_h = {
        l: nc.dram_tensor(f"diag{l}", [P, P], BF16, kind="ExternalInput")
        for l in range(3)
    }
    iota_h = nc.dram_tensor("iota4", [P, SW], BF16, kind="ExternalInput")
    # g_l replicated across all partitions (rank-1 rhs needs matching base
    # partition at 32-aligned offsets)
    grows_h = nc.dram_tensor("grows", [3 * P, P], BF16, kind="ExternalInput")
    # vecs columns: b1,b2,b3, EB1,EB2,EB3, eps
    vecs_h = nc.dram_tensor("vecs", [P, 7], F32, kind="ExternalInput")
    # onesh columns (bf16): [-1/H, +1/H]
    onesh_h = nc.dram_tensor("onesh", [P, 2], BF16, kind="ExternalInput")
    out_h = nc.dram_tensor("out", [NST * P, SW], BF16, kind="ExternalOutput")

    sel_gps = int(os.environ.get("KERNEL_SEL_GPS", "0"))  # gpsimd lacks is_equal
    aggs_eng = os.environ.get("KERNEL_AGGS_ENG", "vector")
    mrs_eng = os.environ.get("KERNEL_MRS_ENG", "vector")
    zsq_gps = os.environ.get("KERNEL_ZSQ_GPS", "0") == "1"
    n_st = int(os.environ.get("KERNEL_NST", str(NST)))

    with tile_mod.TileContext(nc) as tc:
        with (
            tc.tile_pool(name="const", bufs=1) as cpool,
            tc.tile_pool(name="ed", bufs=2) as epool,
            tc.tile_pool(name="xin", bufs=3) as xpool,
            tc.tile_pool(name="sel", bufs=4) as selpool,
            tc.tile_pool(name="work", bufs=3) as wpool,
            tc.tile_pool(name="small", bufs=3) as spool,
            tc.tile_pool(name="pagg", bufs=3, space="PSUM") as pagg,
            tc.tile_pool(name="pz", bufs=2, space="PSUM") as pz,
            tc.tile_pool(name="pzrm", bufs=2, space="PSUM") as pzrm,
            tc.tile_pool(name="pmisc", bufs=1, space="PSUM") as pmisc,
        ):
            identB = cpool.tile([P, P], BF16)
            make_identity(nc, identB[:])
            iota4 = cpool.tile_from(iota_h[:])
            colst = cpool.tile_from(cols_h[:])
            cols4 = colst.rearrange("p (s t k) -> p s t k", s=NST, t=ST, k=K)
            W = {k: cpool.tile_from(h[:], name=f"w_{k}") for k, h in w_h.items()}
            DG = {l: cpool.tile_from(h[:], name=f"dg_{l}") for l, h in diag_h.items()}
            GR = {
                l: cpool.tile_from(grows_h[l * P : (l + 1) * P, :], name=f"gr_{l}")
                for l in range(3)
            }
            vecs = cpool.tile_from(vecs_h[:])
            onesh = cpool.tile_from(onesh_h[:])
            B = {l: vecs[:, l : l + 1] for l in range(3)}
            EB = {l: vecs[:, 3 + l : 4 + l] for l in range(3)}
            epsap = vecs[:, 6:7]
            half = cpool.tile([P, 1], F32)
            nc.gpsimd.memset(half[:], 0.5)

            def sel_engine(k):
                return nc.gpsimd if (k % K) < sel_gps else nc.vector

            def drain_eng(name):
                return {"act": None, "vector": nc.vector, "gpsimd": nc.gpsimd}[name]

            def layer(st, l, z_ps, out_dtype=BF16):
                """z_ps: [h, SW] pre-activation (no bias) in PSUM, col-major.
                Returns act = ln(EB*exp(g*LN(z+b)) + 0.5) as [h, SW] bf16."""
                zb = wpool.tile([P, SW], BF16, tag="zb")
                nc.scalar.activation(
                    zb[:], z_ps[:], mybir.ActivationFunctionType.Identity,
                    bias=B[l],
                )
                zsq = wpool.tile([P, SW], BF16, tag="zsq")
                zsq_eng = nc.gpsimd if zsq_gps else nc.vector
                zsq_eng.tensor_tensor(
                    zsq[:], zb[:], zb[:], op=mybir.AluOpType.mult
                )
                z_rm = pzrm.tile([P, SW], F32, tag="zrm")
                misc = pmisc.tile([P, SW], F32, tag="misc")
                stats = misc[:, 0:8]      # cols 0:4 = -mu, 4:8 = E[z^2]
                for c in range(ST):
                    cs = slice(c * P, (c + 1) * P)
                    # -mu column
                    nc.tensor.matmul(
                        out=stats[:, c : c + 1], lhsT=zb[:, cs],
                        rhs=onesh[:, 0:1], start=True, stop=True,
                    )
                    # E[z^2] column
                    nc.tensor.matmul(
                        out=stats[:, 4 + c : 5 + c], lhsT=zsq[:, cs],
                        rhs=onesh[:, 1:2], start=True, stop=True,
                    )
                musq = spool.tile([P, 4], F32, tag="musq")
                nc.scalar.activation(
                    musq[:], stats[:, 0:4], mybir.ActivationFunctionType.Square
                )
                var = spool.tile([P, 4], F32, tag="var")
                nc.vector.tensor_tensor(
                    var[:], stats[:, 4:8], musq[:], op=mybir.AluOpType.subtract
                )
                lnv = spool.tile([P, 4], F32, tag="lnv")
                nc.scalar.activation(
                    lnv[:], var[:], mybir.ActivationFunctionType.Ln, bias=epsap
                )
                rstd = spool.tile([P, 4], F32, tag="rstd")
                nc.scalar.activation(
                    rstd[:], lnv[:], mybir.ActivationFunctionType.Exp, scale=-0.5
                )
                # -mu rows: transpose each [128,1] column of -mu onto 32-aligned
                # partitions {0,32,64,96} (matmul base-partition constraint)
                mrs = spool.tile([P, 4], BF16, tag="mrs")
                nc.vector.tensor_copy(mrs[:], stats[:, 0:4])
                # row c lands at partition (c%2)*32, free range 256+(c//2)*128
                for c in range(ST):
                    pb = (c % 2) * 32
                    fb = 256 + (c // 2) * P
                    nc.tensor.matmul(
                        out=misc[pb : pb + 1, fb : fb + P],
                        lhsT=mrs[:, c : c + 1], rhs=identB[:],
                        start=True, stop=True,
                    )
                mrsT = spool.tile([33, 2 * P], BF16, tag="mrsT")
                me = drain_eng(mrs_eng)
                if me is None:
                    nc.scalar.activation(
                        mrsT[:], misc[0:33, 256:512],
                        mybir.ActivationFunctionType.Copy,
                    )
                else:
                    me.tensor_copy(mrsT[:], misc[0:33, 256:512])
                e_rm = wpool.tile([P, SW], BF16, tag="erm")
                for c in range(ST):
                    pb = (c % 2) * 32
                    fb = (c // 2) * P
                    cs = slice(c * P, (c + 1) * P)
                    # forward transpose with gamma + rank-1 -mu*g, as one
                    # bank-contiguous accumulation group (a start=True matmul
                    # clears has_written for the whole PSUM bank, so no other
                    # group targeting this bank may open in between)
                    nc.tensor.matmul(
                        out=z_rm[:, cs], lhsT=zb[:, cs], rhs=DG[l][:],
                        start=True, stop=False,
                    )
                    nc.tensor.matmul(
                        out=z_rm[:, cs], lhsT=mrsT[pb : pb + 1, fb : fb + P],
                        rhs=GR[l][pb : pb + 1, :], start=False, stop=True,
                    )
                    # normalize + exp: e = exp((g*z - g*mu) * rstd) per-node
                    nc.scalar.activation(
                        e_rm[:, cs], z_rm[:, cs],
                        mybir.ActivationFunctionType.Exp,
                        scale=rstd[:, c : c + 1],
                    )
                e_ps = pagg.tile([P, SW], F32, tag="agg")
                for c in range(ST):
                    cs = slice(c * P, (c + 1) * P)
                    nc.tensor.matmul(
                        out=e_ps[:, cs], lhsT=e_rm[:, cs], rhs=identB[:],
                        start=True, stop=True,
                    )
                act = wpool.tile([P, SW], out_dtype, tag="act")
                nc.scalar.activation(
                    act[:], e_ps[:], mybir.ActivationFunctionType.Ln,
                    bias=half[:, 0:1], scale=EB[l],
                )
                return act

            for st in range(n_st):
                ed = epool.tile([P, ST * K * P], BF16, tag="ed")
                nc.sync.dma_start(out=ed[:], in_=ed_h[st * P : (st + 1) * P, :])
                xt = xpool.tile([P, SW], BF16, tag="xt")
                nc.sync.dma_start(out=xt[:], in_=xt_h[st * P : (st + 1) * P, :])

                aggP = pagg.tile([P, SW], F32, tag="agg")
                # build all K one-hot masks first, then issue matmuls t-major
                # so each tile's PSUM accumulation group is bank-contiguous
                sels = []
                for k in range(K):
                    sel = selpool.tile([P, SW], BF16, tag=f"sel{k}", bufs=2)
                    sel3 = sel.rearrange("p (t n) -> p t n", t=ST, n=P)
                    cin = cols4[:, st, :, k].unsqueeze(2).to_broadcast([P, ST, P])
                    iin = iota4.rearrange("p (t n) -> p t n", t=ST, n=P)
                    sel_engine(k).tensor_tensor(
                        sel3, cin, iin, op=mybir.AluOpType.is_equal
                    )
                    sels.append(sel3)
                for t in range(ST):
                    for k in range(K):
                        nc.tensor.matmul(
                            out=aggP[:, t * P : (t + 1) * P],
                            lhsT=ed[:, (t * K + k) * P : (t * K + k + 1) * P],
                            rhs=sels[k][:, t, :],
                            start=(k == 0), stop=(k == K - 1),
                        )
                aggS = wpool.tile([P, SW], BF16, tag="aggS")
                ae = drain_eng(aggs_eng)
                if ae is None:
                    nc.scalar.activation(
                        aggS[:], aggP[:], mybir.ActivationFunctionType.Copy
                    )
                else:
                    ae.tensor_copy(aggS[:], aggP[:])

                z1 = pz.tile([P, SW], F32, tag="z")
                nc.tensor.matmul(out=z1[:], lhsT=W["w1a"][:], rhs=xt[:], start=True, stop=False)
                nc.tensor.matmul(out=z1[:], lhsT=W["w1b"][:], rhs=aggS[:], start=False, stop=True)
                a1 = layer(st, 0, z1)

                z2 = pz.tile([P, SW], F32, tag="z")
                nc.tensor.matmul(out=z2[:], lhsT=W["w2"][:], rhs=a1[:], start=True, stop=True)
                a2 = layer(st, 1, z2)

                z3 = pz.tile([P, SW], F32, tag="z")
                nc.tensor.matmul(out=z3[:], lhsT=W["w3"][:], rhs=a2[:], start=True, stop=True)
                a3 = layer(st, 2, z3, out_dtype=BF16)
                nc.sync.dma_start(out=out_h[st * P : (st + 1) * P, :], in_=a3[:])

    if not nc.is_finalized():
        nc.finalize()
    return nc


def kernel(
    x, edge_index, edge_attr,
    W1, b1, g1, be1, W2, b2, g2, be2, W3, b3, g3, be3,
):
    global LAST_RESULT
    W1 = np.asarray(W1, np.float32)
    W2 = np.asarray(W2, np.float32)
    W3 = np.asarray(W3, np.float32)

    K, per_core = _host_prep(x, edge_index, edge_attr)
    nc = _build_program(K)

    gs = [np.asarray(g, np.float32) for g in (g1, g2, g3)]
    bes = [np.asarray(b, np.float32) for b in (be1, be2, be3)]
    bs = [np.asarray(b, np.float32) for b in (b1, b2, b3)]
    vecs = np.stack(bs + [0.5 * np.exp(b) for b in bes] + [np.full(P, 1e-5, np.float32)], axis=1)
    grows = np.concatenate(
        [np.broadcast_to(g, (P, P)) for g in gs], axis=0
    )  # [3*P, P], g_l on every partition
    onesh = np.stack([np.full(P, -1.0 / H, np.float32), np.full(P, 1.0 / H, np.float32)], axis=1)
    shared = {
        "w1a": np.ascontiguousarray(W1[:P]).astype(ml_dtypes.bfloat16),
        "w1b": np.ascontiguousarray(W1[P:]).astype(ml_dtypes.bfloat16),
        "w2": W2.astype(ml_dtypes.bfloat16),
        "w3": W3.astype(ml_dtypes.bfloat16),
        "vecs": np.ascontiguousarray(vecs),
        "grows": grows.astype(ml_dtypes.bfloat16),
        "onesh": onesh.astype(ml_dtypes.bfloat16),
        "iota4": np.ascontiguousarray(
            np.tile(np.arange(P, dtype=np.float32), (P, ST))
        ).astype(ml_dtypes.bfloat16),
    }
    for l in range(3):
        shared[f"diag{l}"] = np.diag(gs[l]).astype(ml_dtypes.bfloat16)
    in_maps = [
        {"edges": ed_c, "cols": col_c, "xt": xt_c, **shared}
        for (ed_c, col_c, xt_c) in per_core
    ]

    trace = bool(int(os.environ.get("KERNEL_TRACE", "0")))
    res = run_bass_kernel_spmd(nc, in_maps, core_ids=list(range(NC)), trace=trace)
    LAST_RESULT = res

    out = np.concatenate(
        [
            np.asarray(r["out"], dtype=np.float32)
            .reshape(NST, P, ST, P)
            .transpose(0, 2, 3, 1)
            .reshape(NPC, H)
            for r in res.results
        ],
        axis=0,
    )
    return np.ascontiguousarray(out[:N])


# revision 27
# speedup vs baseline: 1.1581x; 1.1581x over previous
"""Trainium2 Bass kernel for nn_NodeModel (GNN message passing + 3-layer node MLP).

v2 strategy (node-parallel, 8 cores, 512-node supertiles):
  - Host: sort edges by destination, bucket into 128-node tiles, pad each
    tile's edge list to K chunks of 128 edges. 100 tiles/core, grouped into
    25 supertiles of 4 tiles (512 nodes).
  - Device per supertile:
      agg:   one-hot via DVE/GPSIMD is_equal built [128,512]-wide (4 chunks
             per op via strided/broadcast APs), matmul-accumulated per tile.
      MLP:   z computed col-major [h, n] with W-stationary 512-wide matmuls.
             LayerNorm stats on the PE: per-chunk matmuls with zb/zsq as the
             stationary operand against +-ones/H vectors give -mu and E[z^2]
             as [node,1] PSUM columns; small-ops run on [128,4] tiles.
             Per-node normalize is fused into a per-chunk ACT Exp
             (scale=rstd).  gamma is folded into the forward transpose as a
             diag(g) rhs; the -mu*rstd x g term is added by a rank-1 matmul;
             beta is folded into the final Ln via per-partition EB=0.5*e^be
             scale: act_next = ln(EB * exp(g*(z-mu)*rstd) + 0.5) == ssp out.
  - Output returned bf16 from device, cast to f32 on host.
"""

import os
import sys

import numpy as np

sys.path.insert(0, "/opt/trn_rl_repo")

import bass_rust as _bass_rust
import ml_dtypes

from concourse import bacc, bass, hw_specs, mybir
from concourse import tile as tile_mod
from concourse.bass_utils import run_bass_kernel_spmd
from concourse.masks import make_identity


class _Bacc(bacc.Bacc):
    """Bacc with the ACT table chooser pinned to natural_log_exp_and_others
    (holds Ln+Exp+Identity+Copy+Square), avoiding ~1.3us table swaps."""

    def insert_act_table_loads(self):
        has_activation = any(
            isinstance(i, mybir.InstActivation)
            for b in self.main_func.blocks
            for i in b.instructions
        )
        if not has_activation:
            return
        keep = "natural_log_exp_and_others"
        tables = [
            (n, (s if n == keep else set()))
            for n, s in hw_specs.get_activation_tables(self.m.arch).items()
        ]
        _bass_rust.insert_act_table_loads(self, tables)


N, E, H = 100000, 600000, 128
NC = 8
P = 128
TPC = 100                # 128-node tiles per core
ST = 4                   # tiles per supertile
NST = TPC // ST          # supertiles per core (25)
SW = ST * P              # supertile width in nodes (512)
NPC = TPC * P            # nodes per core (12800)
NPAD = NPC * NC          # padded node count (102400)
NT = NPAD // P           # total node tiles (800)

F32 = mybir.dt.float32
BF16 = mybir.dt.bfloat16

LAST_RESULT = None  # BassKernelResults of the most recent run (for profiling)


def _host_prep(x, edge_index, edge_attr):
    col = np.asarray(edge_index)[1].astype(np.int64)
    ea = np.ascontiguousarray(np.asarray(edge_attr, dtype=np.float32))

    # Degree-balanced node permutation: snake-deal nodes (sorted by degree)
    # into the NT tiles so per-tile edge counts are nearly equal -> smaller K.
    deg = np.bincount(col, minlength=NPAD)
    by_deg = np.argsort(-deg, kind="stable")
    r = np.arange(NPAD)
    b = r % (2 * NT)
    bin_of_rank = np.where(b < NT, b, 2 * NT - 1 - b)
    order_by_bin = np.argsort(bin_of_rank, kind="stable")
    perm_old_by_new = by_deg[order_by_bin]          # new node id -> old node id
    new_of_old = np.empty(NPAD, np.int64)
    new_of_old[perm_old_by_new] = r
    col = new_of_old[col]

    order = np.argsort(col, kind="stable")
    col_s = col[order]
    tile_of = col_s >> 7
    counts = np.bincount(tile_of, minlength=NT)
    K = int(np.ceil(counts.max() / P))
    S = K * P
    starts = np.zeros(NT + 1, np.int64)
    starts[1:] = np.cumsum(counts)
    pos = np.arange(E) - starts[tile_of]
    slot = tile_of * S + pos
    slot_edge = np.zeros(NT * S, np.int64)
    slot_edge[slot] = order
    col_local = np.full(NT * S, 128.0, np.float32)
    col_local[slot] = (col_s & 127).astype(np.float32)
    payload = ea[slot_edge]  # [NT*S, H]

    x_pad = np.zeros((NPAD, H), np.float32)
    x_pad[new_of_old[:N]] = np.asarray(x, dtype=np.float32)

    per_core = []
    for c in range(NC):
        r0, r1 = c * TPC * S, (c + 1) * TPC * S
        # edges: [NST*P, ST*K*P] bf16; row = st*128+e, col = (t*K+k)*128+h
        ed_c = np.ascontiguousarray(
            payload[r0:r1]
            .reshape(NST, ST, K, P, H)
            .transpose(0, 3, 1, 2, 4)
            .reshape(NST * P, ST * K * H)
            .astype(ml_dtypes.bfloat16)
        )
        # cols: [P, NST*ST*K] bf16; col index = st*ST*K + t*K + k
        col_c = np.ascontiguousarray(
            col_local[r0:r1]
            .reshape(NST, ST, K, P)
            .transpose(3, 0, 1, 2)
            .reshape(P, NST * ST * K)
            .astype(ml_dtypes.bfloat16)
        )
        # xt: [NST*P, SW] bf16 col-major per supertile; row st*128+h, col t*128+n
        xt_c = np.ascontiguousarray(
            x_pad[c * NPC : (c + 1) * NPC]
            .reshape(NST, ST, P, H)
            .transpose(0, 3, 1, 2)
            .reshape(NST * P, SW)
            .astype(ml_dtypes.bfloat16)
        )
        per_core.append((ed_c, col_c, xt_c))
    return K, per_core, perm_old_by_new


def _build_program(K, fast_g):
    nc = _Bacc("TRN2", target_bir_lowering=False, debug=False, num_devices=NC)

    ed_h = nc.dram_tensor("edges", [NST * P, ST * K * P], BF16, kind="ExternalInput")
    cols_h = nc.dram_tensor("cols", [P, NST * ST * K], BF16, kind="ExternalInput")
    xt_h = nc.dram_tensor("xt", [NST * P, SW], BF16, kind="ExternalInput")
    w_h = {
        name: nc.dram_tensor(name, [P, P], BF16, kind="ExternalInput")
        for name in ("w1a", "w1b", "w2", "w3")
    }
    diag_h = {
        l: nc.dram_tensor(f"diag{l}", [P, P], BF16, kind="ExternalInput")
        for l in range(3)
    }
    iota_h = nc.dram_tensor("iota4", [P, SW], BF16, kind="ExternalInput")
    # g_l replicated across all partitions (rank-1 rhs needs matching base
    # partition at 32-aligned offsets)
    grows_h = nc.dram_tensor("grows", [3 * P, P], BF16, kind="ExternalInput")
    # vecs columns: b1,b2,b3, EB1,EB2,EB3, eps
    vecs_h = nc.dram_tensor("vecs", [P, 7], F32, kind="ExternalInput")
    # onesh columns (bf16): [-1/H, +1/H]
    onesh_h = nc.dram_tensor("onesh", [P, 2], BF16, kind="ExternalInput")
    out_h = nc.dram_tensor("out", [NST * P, SW], BF16, kind="ExternalOutput")

    sel_gps = int(os.environ.get("KERNEL_SEL_GPS", "0"))  # gpsimd lacks is_equal
    aggs_eng = os.environ.get("KERNEL_AGGS_ENG", "vector")
    mrs_eng = os.environ.get("KERNEL_MRS_ENG", "vector")
    zsq_gps = os.environ.get("KERNEL_ZSQ_GPS", "0") == "1"
    apply_stt = int(os.environ.get("KERNEL_APPLY_STT", "2"))  # chunks on DVE
    n_st = int(os.environ.get("KERNEL_NST", str(NST)))

    with tile_mod.TileContext(nc) as tc:
        with (
            tc.tile_pool(name="const", bufs=1) as cpool,
            tc.tile_pool(name="ed", bufs=2) as epool,
            tc.tile_pool(name="xin", bufs=3) as xpool,
            tc.tile_pool(name="sel", bufs=4) as selpool,
            tc.tile_pool(name="work", bufs=3) as wpool,
            tc.tile_pool(name="small", bufs=3) as spool,
            tc.tile_pool(name="pagg", bufs=3, space="PSUM") as pagg,
            tc.tile_pool(name="pz", bufs=2, space="PSUM") as pz,
            tc.tile_pool(name="pzrm", bufs=2, space="PSUM") as pzrm,
            tc.tile_pool(name="pmisc", bufs=1, space="PSUM") as pmisc,
        ):
            identB = cpool.tile([P, P], BF16)
            make_identity(nc, identB[:])
            iota4 = cpool.tile_from(iota_h[:])
            colst = cpool.tile_from(cols_h[:])
            cols4 = colst.rearrange("p (s t k) -> p s t k", s=NST, t=ST, k=K)
            W = {k: cpool.tile_from(h[:], name=f"w_{k}") for k, h in w_h.items()}
            DG = {l: cpool.tile_from(h[:], name=f"dg_{l}") for l, h in diag_h.items()}
            GR = {
                l: cpool.tile_from(grows_h[l * P : (l + 1) * P, :], name=f"gr_{l}")
                for l in range(3)
            }
            vecs = cpool.tile_from(vecs_h[:])
            onesh = cpool.tile_from(onesh_h[:])
            B = {l: vecs[:, l : l + 1] for l in range(3)}
            EB = {l: vecs[:, 3 + l : 4 + l] for l in range(3)}
            epsap = vecs[:, 6:7]
            half = cpool.tile([P, 1], F32)
            nc.gpsimd.memset(half[:], 0.5)

            def sel_engine(k):
                return nc.gpsimd if (k % K) < sel_gps else nc.vector

            def drain_eng(name):
                return {"act": None, "vector": nc.vector, "gpsimd": nc.gpsimd}[name]

            def layer(st, l, z_ps, out_dtype=BF16):
                """z_ps: [h, SW] pre-activation (no bias) in PSUM, col-major.
                Returns act = ln(EB*exp(g*LN(z+b)) + 0.5) as [h, SW] bf16."""
                zb = wpool.tile([P, SW], BF16, tag="zb")
                nc.scalar.activation(
                    zb[:], z_ps[:], mybir.ActivationFunctionType.Identity,
                    bias=B[l],
                )
                zsq = wpool.tile([P, SW], BF16, tag="zsq")
                zsq_eng = nc.gpsimd if zsq_gps else nc.vector
                zsq_eng.tensor_tensor(
                    zsq[:], zb[:], zb[:], op=mybir.AluOpType.mult
                )
                z_rm = pzrm.tile([P, SW], F32, tag="zrm")
                misc = pmisc.tile([P, SW], F32, tag="misc")
                stats = misc[:, 0:8]      # cols 0:4 = -mu, 4:8 = E[z^2]
                for c in range(ST):
                    cs = slice(c * P, (c + 1) * P)
                    # -mu column
                    nc.tensor.matmul(
                        out=stats[:, c : c + 1], lhsT=zb[:, cs],
                        rhs=onesh[:, 0:1], start=True, stop=True,
                    )
                    # E[z^2] column
                    nc.tensor.matmul(
                        out=stats[:, 4 + c : 5 + c], lhsT=zsq[:, cs],
                        rhs=onesh[:, 1:2], start=True, stop=True,
                    )
                musq = spool.tile([P, 4], F32, tag="musq")
                nc.scalar.activation(
                    musq[:], stats[:, 0:4], mybir.ActivationFunctionType.Square
                )
                var = spool.tile([P, 4], F32, tag="var")
                nc.vector.tensor_tensor(
                    var[:], stats[:, 4:8], musq[:], op=mybir.AluOpType.subtract
                )
                lnv = spool.tile([P, 4], F32, tag="lnv")
                nc.scalar.activation(
                    lnv[:], var[:], mybir.ActivationFunctionType.Ln, bias=epsap
                )
                rstd = spool.tile([P, 4], F32, tag="rstd")
                nc.scalar.activation(
                    rstd[:], lnv[:], mybir.ActivationFunctionType.Exp, scale=-0.5
                )
                e_rm = wpool.tile([P, SW], BF16, tag="erm")
                if fast_g:
                    # g == 1: -mu*rstd is purely per-node -> it is the Exp
                    # bias / stt addend directly; no rank-1, no mrsT.
                    mrs = spool.tile([P, 4], F32, tag="mrs")
                    nc.vector.tensor_tensor(
                        mrs[:], stats[:, 0:4], rstd[:], op=mybir.AluOpType.mult
                    )
                    v_rm = wpool.tile([P, SW], BF16, tag="vrm")
                    for c in range(ST):
                        cs = slice(c * P, (c + 1) * P)
                        nc.tensor.matmul(
                            out=z_rm[:, cs], lhsT=zb[:, cs], rhs=DG[l][:],
                            start=True, stop=True,
                        )
                        if c < apply_stt:
                            # v = z*rstd + mrs on DVE; exp'd below in one op
                            nc.vector.scalar_tensor_tensor(
                                v_rm[:, cs], z_rm[:, cs], rstd[:, c : c + 1],
                                mrs[:, c : c + 1].to_broadcast([P, P]),
                                op0=mybir.AluOpType.mult,
                                op1=mybir.AluOpType.add,
                            )
                        else:
                            nc.scalar.activation(
                                e_rm[:, cs], z_rm[:, cs],
                                mybir.ActivationFunctionType.Exp,
                                bias=mrs[:, c : c + 1],
                                scale=rstd[:, c : c + 1],
                            )
                    if apply_stt > 0:
                        aw = apply_stt * P
                        nc.scalar.activation(
                            e_rm[:, 0:aw], v_rm[:, 0:aw],
                            mybir.ActivationFunctionType.Exp,
                        )
                else:
                    # general gamma: -mu rows transposed onto 32-aligned
                    # partitions {0,32}x{2 col ranges}, then rank-1 (-mu)(x)g
                    mrs = spool.tile([P, 4], BF16, tag="mrs")
                    nc.vector.tensor_copy(mrs[:], stats[:, 0:4])
                    for c in range(ST):
                        pb = (c % 2) * 32
                        fb = 256 + (c // 2) * P
                        nc.tensor.matmul(
                            out=misc[pb : pb + 1, fb : fb + P],
                            lhsT=mrs[:, c : c + 1], rhs=identB[:],
                            start=True, stop=True,
                        )
                    mrsT = spool.tile([33, 2 * P], BF16, tag="mrsT")
                    me = drain_eng(mrs_eng)
                    if me is None:
                        nc.scalar.activation(
                            mrsT[:], misc[0:33, 256:512],
                            mybir.ActivationFunctionType.Copy,
                        )
                    else:
                        me.tensor_copy(mrsT[:], misc[0:33, 256:512])
                    for c in range(ST):
                        pb = (c % 2) * 32
                        fb = (c // 2) * P
                        cs = slice(c * P, (c + 1) * P)
                        # fwd transpose with gamma + rank-1 -mu*g, as one
                        # bank-contiguous accumulation group (a start=True
                        # matmul clears has_written for its whole PSUM bank)
                        nc.tensor.matmul(
                            out=z_rm[:, cs], lhsT=zb[:, cs], rhs=DG[l][:],
                            start=True, stop=False,
                        )
                        nc.tensor.matmul(
                            out=z_rm[:, cs], lhsT=mrsT[pb : pb + 1, fb : fb + P],
                            rhs=GR[l][pb : pb + 1, :], start=False, stop=True,
                        )
                        # normalize + exp: e = exp((g*z - g*mu) * rstd)
                        nc.scalar.activation(
                            e_rm[:, cs], z_rm[:, cs],
                            mybir.ActivationFunctionType.Exp,
                            scale=rstd[:, c : c + 1],
                        )
                e_ps = pagg.tile([P, SW], F32, tag="agg")
                for c in range(ST):
                    cs = slice(c * P, (c + 1) * P)
                    nc.tensor.matmul(
                        out=e_ps[:, cs], lhsT=e_rm[:, cs], rhs=identB[:],
                        start=True, stop=True,
                    )
                act = wpool.tile([P, SW], out_dtype, tag="act")
                nc.scalar.activation(
                    act[:], e_ps[:], mybir.ActivationFunctionType.Ln,
                    bias=half[:, 0:1], scale=EB[l],
                )
                return act

            for st in range(n_st):
                ed = epool.tile([P, ST * K * P], BF16, tag="ed")
                nc.sync.dma_start(out=ed[:], in_=ed_h[st * P : (st + 1) * P, :])
                xt = xpool.tile([P, SW], BF16, tag="xt")
                nc.sync.dma_start(out=xt[:], in_=xt_h[st * P : (st + 1) * P, :])

                aggP = pagg.tile([P, SW], F32, tag="agg")
                # build all K one-hot masks first, then issue matmuls t-major
                # so each tile's PSUM accumulation group is bank-contiguous
                sels = []
                for k in range(K):
                    sel = selpool.tile([P, SW], BF16, tag=f"sel{k}", bufs=2)
                    sel3 = sel.rearrange("p (t n) -> p t n", t=ST, n=P)
                    cin = cols4[:, st, :, k].unsqueeze(2).to_broadcast([P, ST, P])
                    iin = iota4.rearrange("p (t n) -> p t n", t=ST, n=P)
                    sel_engine(k).tensor_tensor(
                        sel3, cin, iin, op=mybir.AluOpType.is_equal
                    )
                    sels.append(sel3)
                for t in range(ST):
                    for k in range(K):
                        nc.tensor.matmul(
                            out=aggP[:, t * P : (t + 1) * P],
                            lhsT=ed[:, (t * K + k) * P : (t * K + k + 1) * P],
                            rhs=sels[k][:, t, :],
                            start=(k == 0), stop=(k == K - 1),
                        )
                aggS = wpool.tile([P, SW], BF16, tag="aggS")
                ae = drain_eng(aggs_eng)
                if ae is None:
                    nc.scalar.activation(
                        aggS[:], aggP[:], mybir.ActivationFunctionType.Copy
                    )
                else:
                    ae.tensor_copy(aggS[:], aggP[:])

                z1 = pz.tile([P, SW], F32, tag="z")
                nc.tensor.matmul(out=z1[:], lhsT=W["w1a"][:], rhs=xt[:], start=True, stop=False)
                nc.tensor.matmul(out=z1[:], lhsT=W["w1b"][:], rhs=aggS[:], start=False, stop=True)
                a1 = layer(st, 0, z1)

                z2 = pz.tile([P, SW], F32, tag="z")
                nc.tensor.matmul(out=z2[:], lhsT=W["w2"][:], rhs=a1[:], start=True, stop=True)
                a2 = layer(st, 1, z2)

                z3 = pz.tile([P, SW], F32, tag="z")
                nc.tensor.matmul(out=z3[:], lhsT=W["w3"][:], rhs=a2[:], start=True, stop=True)
                a3 = layer(st, 2, z3, out_dtype=BF16)
                nc.sync.dma_start(out=out_h[st * P : (st + 1) * P, :], in_=a3[:])

    if not nc.is_finalized():
        nc.finalize()
    return nc


def kernel(
    x, edge_index, edge_attr,
    W1, b1, g1, be1, W2, b2, g2, be2, W3, b3, g3, be3,
):
    global LAST_RESULT
    W1 = np.asarray(W1, np.float32)
    W2 = np.asarray(W2, np.float32)
    W3 = np.asarray(W3, np.float32)

    K, per_core, perm_old_by_new = _host_prep(x, edge_index, edge_attr)
    gs = [np.asarray(g, np.float32) for g in (g1, g2, g3)]
    fast_g = all(np.allclose(g, 1.0) for g in gs)
    nc = _build_program(K, fast_g)
    bes = [np.asarray(b, np.float32) for b in (be1, be2, be3)]
    bs = [np.asarray(b, np.float32) for b in (b1, b2, b3)]
    vecs = np.stack(bs + [0.5 * np.exp(b) for b in bes] + [np.full(P, 1e-5, np.float32)], axis=1)
    grows = np.concatenate(
        [np.broadcast_to(g, (P, P)) for g in gs], axis=0
    )  # [3*P, P], g_l on every partition
    onesh = np.stack([np.full(P, -1.0 / H, np.float32), np.full(P, 1.0 / H, np.float32)], axis=1)
    shared = {
        "w1a": np.ascontiguousarray(W1[:P]).astype(ml_dtypes.bfloat16),
        "w1b": np.ascontiguousarray(W1[P:]).astype(ml_dtypes.bfloat16),
        "w2": W2.astype(ml_dtypes.bfloat16),
        "w3": W3.astype(ml_dtypes.bfloat16),
        "vecs": np.ascontiguousarray(vecs),
        "grows": grows.astype(ml_dtypes.bfloat16),
        "onesh": onesh.astype(ml_dtypes.bfloat16),
        "iota4": np.ascontiguousarray(
            np.tile(np.arange(P, dtype=np.float32), (P, ST))
        ).astype(ml_dtypes.bfloat16),
    }
    for l in range(3):
        shared[f"diag{l}"] = np.diag(gs[l]).astype(ml_dtypes.bfloat16)
    in_maps = [
        {"edges": ed_c, "cols": col_c, "xt": xt_c, **shared}
        for (ed_c, col_c, xt_c) in per_core
    ]

    trace = bool(int(os.environ.get("KERNEL_TRACE", "0")))
    res = run_bass_kernel_spmd(nc, in_maps, core_ids=list(range(NC)), trace=trace)
    LAST_RESULT = res

    out = np.concatenate(
        [
            np.asarray(r["out"], dtype=np.float32)
            .reshape(NST, P, ST, P)
            .transpose(0, 2, 3, 1)
            .reshape(NPC, H)
            for r in res.results
        ],
        axis=0,
    )
    y = np.empty((N, H), np.float32)
    keep = perm_old_by_new < N
    y[perm_old_by_new[keep]] = out[keep]
    return y


# revision 28
# speedup vs baseline: 1.1591x; 1.0009x over previous
"""Trainium2 Bass kernel for nn_NodeModel (GNN message passing + 3-layer node MLP).

v2 strategy (node-parallel, 8 cores, 512-node supertiles):
  - Host: sort edges by destination, bucket into 128-node tiles, pad each
    tile's edge list to K chunks of 128 edges. 100 tiles/core, grouped into
    25 supertiles of 4 tiles (512 nodes).
  - Device per supertile:
      agg:   one-hot via DVE/GPSIMD is_equal built [128,512]-wide (4 chunks
             per op via strided/broadcast APs), matmul-accumulated per tile.
      MLP:   z computed col-major [h, n] with W-stationary 512-wide matmuls.
             LayerNorm stats on the PE: per-chunk matmuls with zb/zsq as the
             stationary operand against +-ones/H vectors give -mu and E[z^2]
             as [node,1] PSUM columns; small-ops run on [128,4] tiles.
             Per-node normalize is fused into a per-chunk ACT Exp
             (scale=rstd).  gamma is folded into the forward transpose as a
             diag(g) rhs; the -mu*rstd x g term is added by a rank-1 matmul;
             beta is folded into the final Ln via per-partition EB=0.5*e^be
             scale: act_next = ln(EB * exp(g*(z-mu)*rstd) + 0.5) == ssp out.
  - Output returned bf16 from device, cast to f32 on host.
"""

import os
import sys

import numpy as np

sys.path.insert(0, "/opt/trn_rl_repo")

import bass_rust as _bass_rust
import ml_dtypes

from concourse import bacc, bass, hw_specs, mybir
from concourse import tile as tile_mod
from concourse.bass_utils import run_bass_kernel_spmd
from concourse.masks import make_identity


class _Bacc(bacc.Bacc):
    """Bacc with the ACT table chooser pinned to natural_log_exp_and_others
    (holds Ln+Exp+Identity+Copy+Square), avoiding ~1.3us table swaps."""

    def insert_act_table_loads(self):
        has_activation = any(
            isinstance(i, mybir.InstActivation)
            for b in self.main_func.blocks
            for i in b.instructions
        )
        if not has_activation:
            return
        keep = "natural_log_exp_and_others"
        tables = [
            (n, (s if n == keep else set()))
            for n, s in hw_specs.get_activation_tables(self.m.arch).items()
        ]
        _bass_rust.insert_act_table_loads(self, tables)


N, E, H = 100000, 600000, 128
NC = 8
P = 128
TPC = 100                # 128-node tiles per core
ST = 4                   # tiles per supertile
NST = TPC // ST          # supertiles per core (25)
SW = ST * P              # supertile width in nodes (512)
NPC = TPC * P            # nodes per core (12800)
NPAD = NPC * NC          # padded node count (102400)
NT = NPAD // P           # total node tiles (800)

F32 = mybir.dt.float32
BF16 = mybir.dt.bfloat16

LAST_RESULT = None  # BassKernelResults of the most recent run (for profiling)


def _host_prep(x, edge_index, edge_attr):
    col = np.asarray(edge_index)[1].astype(np.int64)
    ea = np.ascontiguousarray(np.asarray(edge_attr, dtype=np.float32))

    # Degree-balanced node permutation: snake-deal nodes (sorted by degree)
    # into the NT tiles so per-tile edge counts are nearly equal -> smaller K.
    deg = np.bincount(col, minlength=NPAD)
    by_deg = np.argsort(-deg, kind="stable")
    r = np.arange(NPAD)
    b = r % (2 * NT)
    bin_of_rank = np.where(b < NT, b, 2 * NT - 1 - b)
    order_by_bin = np.argsort(bin_of_rank, kind="stable")
    perm_old_by_new = by_deg[order_by_bin]          # new node id -> old node id
    new_of_old = np.empty(NPAD, np.int64)
    new_of_old[perm_old_by_new] = r
    col = new_of_old[col]

    order = np.argsort(col, kind="stable")
    col_s = col[order]
    tile_of = col_s >> 7
    counts = np.bincount(tile_of, minlength=NT)
    K = int(np.ceil(counts.max() / P))
    S = K * P
    starts = np.zeros(NT + 1, np.int64)
    starts[1:] = np.cumsum(counts)
    pos = np.arange(E) - starts[tile_of]
    slot = tile_of * S + pos
    slot_edge = np.zeros(NT * S, np.int64)
    slot_edge[slot] = order
    col_local = np.full(NT * S, 128.0, np.float32)
    col_local[slot] = (col_s & 127).astype(np.float32)
    payload = ea[slot_edge]  # [NT*S, H]

    x_pad = np.zeros((NPAD, H), np.float32)
    x_pad[new_of_old[:N]] = np.asarray(x, dtype=np.float32)

    per_core = []
    for c in range(NC):
        r0, r1 = c * TPC * S, (c + 1) * TPC * S
        # edges: [NST*P, ST*K*P] bf16; row = st*128+e, col = (t*K+k)*128+h
        ed_c = np.ascontiguousarray(
            payload[r0:r1]
            .reshape(NST, ST, K, P, H)
            .transpose(0, 3, 1, 2, 4)
            .reshape(NST * P, ST * K * H)
            .astype(ml_dtypes.bfloat16)
        )
        # cols: [P, NST*ST*K] bf16; col index = st*ST*K + t*K + k
        col_c = np.ascontiguousarray(
            col_local[r0:r1]
            .reshape(NST, ST, K, P)
            .transpose(3, 0, 1, 2)
            .reshape(P, NST * ST * K)
            .astype(ml_dtypes.bfloat16)
        )
        # xt: [NST*P, SW] bf16 col-major per supertile; row st*128+h, col t*128+n
        xt_c = np.ascontiguousarray(
            x_pad[c * NPC : (c + 1) * NPC]
            .reshape(NST, ST, P, H)
            .transpose(0, 3, 1, 2)
            .reshape(NST * P, SW)
            .astype(ml_dtypes.bfloat16)
        )
        per_core.append((ed_c, col_c, xt_c))
    return K, per_core, perm_old_by_new


def _build_program(K, fast_g):
    nc = _Bacc("TRN2", target_bir_lowering=False, debug=False, num_devices=NC)

    ed_h = nc.dram_tensor("edges", [NST * P, ST * K * P], BF16, kind="ExternalInput")
    cols_h = nc.dram_tensor("cols", [P, NST * ST * K], BF16, kind="ExternalInput")
    xt_h = nc.dram_tensor("xt", [NST * P, SW], BF16, kind="ExternalInput")
    w_h = {
        name: nc.dram_tensor(name, [P, P], BF16, kind="ExternalInput")
        for name in ("w1a", "w1b", "w2", "w3")
    }
    diag_h = {
        l: nc.dram_tensor(f"diag{l}", [P, P], BF16, kind="ExternalInput")
        for l in range(3)
    }
    iota_h = nc.dram_tensor("iota4", [P, SW], BF16, kind="ExternalInput")
    # g_l replicated across all partitions (rank-1 rhs needs matching base
    # partition at 32-aligned offsets)
    grows_h = nc.dram_tensor("grows", [3 * P, P], BF16, kind="ExternalInput")
    # vecs columns: b1,b2,b3, EB1,EB2,EB3, eps
    vecs_h = nc.dram_tensor("vecs", [P, 7], F32, kind="ExternalInput")
    # onesh columns (bf16): [-1/H, +1/H]
    onesh_h = nc.dram_tensor("onesh", [P, 2], BF16, kind="ExternalInput")
    out_h = nc.dram_tensor("out", [NST * P, SW], BF16, kind="ExternalOutput")

    sel_gps = int(os.environ.get("KERNEL_SEL_GPS", "0"))  # gpsimd lacks is_equal
    aggs_eng = os.environ.get("KERNEL_AGGS_ENG", "vector")
    mrs_eng = os.environ.get("KERNEL_MRS_ENG", "vector")
    zsq_gps = os.environ.get("KERNEL_ZSQ_GPS", "0") == "1"
    apply_stt = int(os.environ.get("KERNEL_APPLY_STT", "2"))  # chunks on DVE
    n_st = int(os.environ.get("KERNEL_NST", str(NST)))

    with tile_mod.TileContext(nc) as tc:
        with (
            tc.tile_pool(name="const", bufs=1) as cpool,
            tc.tile_pool(name="ed", bufs=2) as epool,
            tc.tile_pool(name="xin", bufs=3) as xpool,
            tc.tile_pool(name="sel", bufs=4) as selpool,
            tc.tile_pool(name="work", bufs=3) as wpool,
            tc.tile_pool(name="small", bufs=3) as spool,
            tc.tile_pool(name="psum", bufs=8, space="PSUM") as ppool,
        ):
            identB = cpool.tile([P, P], BF16)
            make_identity(nc, identB[:])
            iota4 = cpool.tile_from(iota_h[:])
            colst = cpool.tile_from(cols_h[:])
            cols4 = colst.rearrange("p (s t k) -> p s t k", s=NST, t=ST, k=K)
            W = {k: cpool.tile_from(h[:], name=f"w_{k}") for k, h in w_h.items()}
            DG = {l: cpool.tile_from(h[:], name=f"dg_{l}") for l, h in diag_h.items()}
            GR = {
                l: cpool.tile_from(grows_h[l * P : (l + 1) * P, :], name=f"gr_{l}")
                for l in range(3)
            }
            vecs = cpool.tile_from(vecs_h[:])
            onesh = cpool.tile_from(onesh_h[:])
            B = {l: vecs[:, l : l + 1] for l in range(3)}
            EB = {l: vecs[:, 3 + l : 4 + l] for l in range(3)}
            epsap = vecs[:, 6:7]
            half = cpool.tile([P, 1], F32)
            nc.gpsimd.memset(half[:], 0.5)

            def sel_engine(k):
                return nc.gpsimd if (k % K) < sel_gps else nc.vector

            def drain_eng(name):
                return {"act": None, "vector": nc.vector, "gpsimd": nc.gpsimd}[name]

            def layer(st, l, z_ps, out_dtype=BF16):
                """z_ps: [h, SW] pre-activation (no bias) in PSUM, col-major.
                Returns act = ln(EB*exp(g*LN(z+b)) + 0.5) as [h, SW] bf16."""
                zb = wpool.tile([P, SW], BF16, tag="zb")
                nc.scalar.activation(
                    zb[:], z_ps[:], mybir.ActivationFunctionType.Identity,
                    bias=B[l],
                )
                zsq = wpool.tile([P, SW], BF16, tag="zsq")
                zsq_eng = nc.gpsimd if zsq_gps else nc.vector
                zsq_eng.tensor_tensor(
                    zsq[:], zb[:], zb[:], op=mybir.AluOpType.mult
                )
                z_rm = ppool.tile([P, SW], F32, tag="ps")
                misc = ppool.tile([P, SW], F32, tag="ps")
                stats = misc[:, 0:8]      # cols 0:4 = -mu, 4:8 = E[z^2]
                for c in range(ST):
                    cs = slice(c * P, (c + 1) * P)
                    # -mu column
                    nc.tensor.matmul(
                        out=stats[:, c : c + 1], lhsT=zb[:, cs],
                        rhs=onesh[:, 0:1], start=True, stop=True,
                    )
                    # E[z^2] column
                    nc.tensor.matmul(
                        out=stats[:, 4 + c : 5 + c], lhsT=zsq[:, cs],
                        rhs=onesh[:, 1:2], start=True, stop=True,
                    )
                musq = spool.tile([P, 4], F32, tag="musq")
                nc.scalar.activation(
                    musq[:], stats[:, 0:4], mybir.ActivationFunctionType.Square
                )
                var = spool.tile([P, 4], F32, tag="var")
                nc.vector.tensor_tensor(
                    var[:], stats[:, 4:8], musq[:], op=mybir.AluOpType.subtract
                )
                lnv = spool.tile([P, 4], F32, tag="lnv")
                nc.scalar.activation(
                    lnv[:], var[:], mybir.ActivationFunctionType.Ln, bias=epsap
                )
                rstd = spool.tile([P, 4], F32, tag="rstd")
                nc.scalar.activation(
                    rstd[:], lnv[:], mybir.ActivationFunctionType.Exp, scale=-0.5
                )
                e_rm = wpool.tile([P, SW], BF16, tag="erm")
                if fast_g:
                    # g == 1: -mu*rstd is purely per-node -> it is the Exp
                    # bias / stt addend directly; no rank-1, no mrsT.
                    mrs = spool.tile([P, 4], F32, tag="mrs")
                    nc.vector.tensor_tensor(
                        mrs[:], stats[:, 0:4], rstd[:], op=mybir.AluOpType.mult
                    )
                    v_rm = wpool.tile([P, SW], BF16, tag="vrm")
                    for c in range(ST):
                        cs = slice(c * P, (c + 1) * P)
                        nc.tensor.matmul(
                            out=z_rm[:, cs], lhsT=zb[:, cs], rhs=DG[l][:],
                            start=True, stop=True,
                        )
                        if c < apply_stt:
                            # v = z*rstd + mrs on DVE; exp'd below in one op
                            nc.vector.scalar_tensor_tensor(
                                v_rm[:, cs], z_rm[:, cs], rstd[:, c : c + 1],
                                mrs[:, c : c + 1].to_broadcast([P, P]),
                                op0=mybir.AluOpType.mult,
                                op1=mybir.AluOpType.add,
                            )
                        else:
                            nc.scalar.activation(
                                e_rm[:, cs], z_rm[:, cs],
                                mybir.ActivationFunctionType.Exp,
                                bias=mrs[:, c : c + 1],
                                scale=rstd[:, c : c + 1],
                            )
                    if apply_stt > 0:
                        aw = apply_stt * P
                        nc.scalar.activation(
                            e_rm[:, 0:aw], v_rm[:, 0:aw],
                            mybir.ActivationFunctionType.Exp,
                        )
                else:
                    # general gamma: -mu rows transposed onto 32-aligned
                    # partitions {0,32}x{2 col ranges}, then rank-1 (-mu)(x)g
                    mrs = spool.tile([P, 4], BF16, tag="mrs")
                    nc.vector.tensor_copy(mrs[:], stats[:, 0:4])
                    for c in range(ST):
                        pb = (c % 2) * 32
                        fb = 256 + (c // 2) * P
                        nc.tensor.matmul(
                            out=misc[pb : pb + 1, fb : fb + P],
                            lhsT=mrs[:, c : c + 1], rhs=identB[:],
                            start=True, stop=True,
                        )
                    mrsT = spool.tile([33, 2 * P], BF16, tag="mrsT")
                    me = drain_eng(mrs_eng)
                    if me is None:
                        nc.scalar.activation(
                            mrsT[:], misc[0:33, 256:512],
                            mybir.ActivationFunctionType.Copy,
                        )
                    else:
                        me.tensor_copy(mrsT[:], misc[0:33, 256:512])
                    for c in range(ST):
                        pb = (c % 2) * 32
                        fb = (c // 2) * P
                        cs = slice(c * P, (c + 1) * P)
                        # fwd transpose with gamma + rank-1 -mu*g, as one
                        # bank-contiguous accumulation group (a start=True
                        # matmul clears has_written for its whole PSUM bank)
                        nc.tensor.matmul(
                            out=z_rm[:, cs], lhsT=zb[:, cs], rhs=DG[l][:],
                            start=True, stop=False,
                        )
                        nc.tensor.matmul(
                            out=z_rm[:, cs], lhsT=mrsT[pb : pb + 1, fb : fb + P],
                            rhs=GR[l][pb : pb + 1, :], start=False, stop=True,
                        )
                        # normalize + exp: e = exp((g*z - g*mu) * rstd)
                        nc.scalar.activation(
                            e_rm[:, cs], z_rm[:, cs],
                            mybir.ActivationFunctionType.Exp,
                            scale=rstd[:, c : c + 1],
                        )
                e_ps = ppool.tile([P, SW], F32, tag="ps")
                for c in range(ST):
                    cs = slice(c * P, (c + 1) * P)
                    nc.tensor.matmul(
                        out=e_ps[:, cs], lhsT=e_rm[:, cs], rhs=identB[:],
                        start=True, stop=True,
                    )
                act = wpool.tile([P, SW], out_dtype, tag="act")
                nc.scalar.activation(
                    act[:], e_ps[:], mybir.ActivationFunctionType.Ln,
                    bias=half[:, 0:1], scale=EB[l],
                )
                return act

            for st in range(n_st):
                ed = epool.tile([P, ST * K * P], BF16, tag="ed")
                nc.sync.dma_start(out=ed[:], in_=ed_h[st * P : (st + 1) * P, :])
                xt = xpool.tile([P, SW], BF16, tag="xt")
                nc.sync.dma_start(out=xt[:], in_=xt_h[st * P : (st + 1) * P, :])

                aggP = ppool.tile([P, SW], F32, tag="ps")
                # build all K one-hot masks first, then issue matmuls t-major
                # so each tile's PSUM accumulation group is bank-contiguous
                sels = []
                for k in range(K):
                    sel = selpool.tile([P, SW], BF16, tag=f"sel{k}", bufs=2)
                    sel3 = sel.rearrange("p (t n) -> p t n", t=ST, n=P)
                    cin = cols4[:, st, :, k].unsqueeze(2).to_broadcast([P, ST, P])
                    iin = iota4.rearrange("p (t n) -> p t n", t=ST, n=P)
                    sel_engine(k).tensor_tensor(
                        sel3, cin, iin, op=mybir.AluOpType.is_equal
                    )
                    sels.append(sel3)
                for t in range(ST):
                    for k in range(K):
                        nc.tensor.matmul(
                            out=aggP[:, t * P : (t + 1) * P],
                            lhsT=ed[:, (t * K + k) * P : (t * K + k + 1) * P],
                            rhs=sels[k][:, t, :],
                            start=(k == 0), stop=(k == K - 1),
                        )
                aggS = wpool.tile([P, SW], BF16, tag="aggS")
                ae = drain_eng(aggs_eng)
                if ae is None:
                    nc.scalar.activation(
                        aggS[:], aggP[:], mybir.ActivationFunctionType.Copy
                    )
                else:
                    ae.tensor_copy(aggS[:], aggP[:])

                z1 = ppool.tile([P, SW], F32, tag="ps")
                nc.tensor.matmul(out=z1[:], lhsT=W["w1a"][:], rhs=xt[:], start=True, stop=False)
                nc.tensor.matmul(out=z1[:], lhsT=W["w1b"][:], rhs=aggS[:], start=False, stop=True)
                a1 = layer(st, 0, z1)

                z2 = ppool.tile([P, SW], F32, tag="ps")
                nc.tensor.matmul(out=z2[:], lhsT=W["w2"][:], rhs=a1[:], start=True, stop=True)
                a2 = layer(st, 1, z2)

                z3 = ppool.tile([P, SW], F32, tag="ps")
                nc.tensor.matmul(out=z3[:], lhsT=W["w3"][:], rhs=a2[:], start=True, stop=True)
                a3 = layer(st, 2, z3, out_dtype=BF16)
                nc.sync.dma_start(out=out_h[st * P : (st + 1) * P, :], in_=a3[:])

    if not nc.is_finalized():
        nc.finalize()
    return nc


def kernel(
    x, edge_index, edge_attr,
    W1, b1, g1, be1, W2, b2, g2, be2, W3, b3, g3, be3,
):
    global LAST_RESULT
    W1 = np.asarray(W1, np.float32)
    W2 = np.asarray(W2, np.float32)
    W3 = np.asarray(W3, np.float32)

    K, per_core, perm_old_by_new = _host_prep(x, edge_index, edge_attr)
    gs = [np.asarray(g, np.float32) for g in (g1, g2, g3)]
    fast_g = all(np.allclose(g, 1.0) for g in gs)
    nc = _build_program(K, fast_g)
    bes = [np.asarray(b, np.float32) for b in (be1, be2, be3)]
    bs = [np.asarray(b, np.float32) for b in (b1, b2, b3)]
    vecs = np.stack(bs + [0.5 * np.exp(b) for b in bes] + [np.full(P, 1e-5, np.float32)], axis=1)
    grows = np.concatenate(
        [np.broadcast_to(g, (P, P)) for g in gs], axis=0
    )  # [3*P, P], g_l on every partition
    onesh = np.stack([np.full(P, -1.0 / H, np.float32), np.full(P, 1.0 / H, np.float32)], axis=1)
    shared = {
        "w1a": np.ascontiguousarray(W1[:P]).astype(ml_dtypes.bfloat16),
        "w1b": np.ascontiguousarray(W1[P:]).astype(ml_dtypes.bfloat16),
        "w2": W2.astype(ml_dtypes.bfloat16),
        "w3": W3.astype(ml_dtypes.bfloat16),
        "vecs": np.ascontiguousarray(vecs),
        "grows": grows.astype(ml_dtypes.bfloat16),
        "onesh": onesh.astype(ml_dtypes.bfloat16),
        "iota4": np.ascontiguousarray(
            np.tile(np.arange(P, dtype=np.float32), (P, ST))
        ).astype(ml_dtypes.bfloat16),
    }
    for l in range(3):
        shared[f"diag{l}"] = np.diag(gs[l]).astype(ml_dtypes.bfloat16)
    in_maps = [
        {"edges": ed_c, "cols": col_c, "xt": xt_c, **shared}
        for (ed_c, col_c, xt_c) in per_core
    ]

    trace = bool(int(os.environ.get("KERNEL_TRACE", "0")))
    res = run_bass_kernel_spmd(nc, in_maps, core_ids=list(range(NC)), trace=trace)
    LAST_RESULT = res

    out = np.concatenate(
        [
            np.asarray(r["out"], dtype=np.float32)
            .reshape(NST, P, ST, P)
            .transpose(0, 2, 3, 1)
            .reshape(NPC, H)
            for r in res.results
        ],
        axis=0,
    )
    y = np.empty((N, H), np.float32)
    keep = perm_old_by_new < N
    y[perm_old_by_new[keep]] = out[keep]
    return y
